# revision 30
# baseline (speedup 1.0000x reference)
"""Trainium2 Bass kernel for sigmoid-projection strictly-causal attention.

Reference computation (B=8, S=2048, D=512, U=512):
    q = sigmoid(x @ Wq); k = sigmoid(x @ Wv); v = sigmoid(x @ Wk)
    score = (q @ k^T) / sqrt(D)                       [S, S]
    mask: strictly causal (key j < query i); row 0 -> zeros
    out = softmax(score) @ v                          [S, U]

Sharding: data-parallel over batch, one batch element per NeuronCore
(8 cores), weights replicated, no collectives.  Full inputs in, full
[B, S, U] output back.

v2 design (vs the 156us baseline):
  - PE warmup: dep-free dummy matmuls at t=0 cover the ~9us DMA/runtime
    startup and flip the HAM clock gate (PE 1.2 -> 2.4 GHz) before real
    work arrives.  The baseline ran its whole phase 1 at half clock.
  - X streams in as bf16 via gpsimd casting DMAs (one per 512-row
    chunk, f32 DRAM -> bf16 SBUF), so the PE transposes for x^T run at
    1 cycle/row instead of f32's 2; four transposes share one PSUM
    bank, evicted by a single DVE copy (casting to fp8 for the
    DoubleRow projections).
  - Projections run as fp8 DoubleRow matmuls; sigmoid evictions read
    [128, 1024] two-bank PSUM pairs (half the ACT instructions).
  - Scores are computed TRANSPOSED (stationary kT block, moving qT
    span), so exp writes P^T straight to SBUF: the baseline's 136 PE
    transposes + 136 DVE copies vanish.  Stream lengths are unchanged
    (512-col chunks).
  - Softmax denominators are per-key-column sums = partition-direction
    sums of P^T: ones-vector matmuls (1-wide stationary, LDWEIGHTS
    ~free) accumulated per query-chunk, spread to partitions by a tiny
    DMA, flipped by one small PE transpose, reciprocal on DVE.
  - PV consumes P^T blocks as stationary, V tiles as moving; rows are
    evicted unnormalized and scaled by 1/denom when their chunk's
    denominator closes, then DMA'd out.
"""

import sys

for _p in ("/opt/trn_rl_repo",):
    if _p not in sys.path:
        sys.path.insert(0, _p)

import numpy as np

B, S, D, U = 8, 2048, 512, 512
P = 128
NCORES = 8
DT = D // P  # 4 d-tiles
UT = U // P  # 4 u-tiles
ST = S // P  # 16 s-tiles
SC = S // 512  # 4 s-chunks
WARMUP_N = 10  # dummy PE matmuls covering startup; tuned via trace

_cache = {}


def _build():
    import ml_dtypes
    import concourse.mybir as mybir
    import concourse.tile as tile
    from concourse import bacc

    f32 = mybir.dt.float32
    bf16 = mybir.dt.bfloat16
    f8 = mybir.dt.float8e4
    AF = mybir.ActivationFunctionType
    DR = mybir.MatmulPerfMode.DoubleRow

    nc = bacc.Bacc("TRN2", target_bir_lowering=False, debug=False,
                   num_devices=NCORES)

    x_ext = nc.dram_tensor("query", [S, D], f32, kind="ExternalInput")
    wq_ext = nc.dram_tensor("Wq", [D, U], f32, kind="ExternalInput")
    wv_ext = nc.dram_tensor("Wv", [D, U], f32, kind="ExternalInput")
    wk_ext = nc.dram_tensor("Wk", [D, U], f32, kind="ExternalInput")
    out_ext = nc.dram_tensor("out", [S, U], f32, kind="ExternalOutput")

    # [sk_p, sq_f] transposed diagonal-block additive mask: keep (0) where
    # key k < query q, else -1e30.  Applied to transposed scores pre-exp.
    maskT_dram = nc.inline_tensor(
        np.where(np.triu(np.ones((P, P), bool), 1), 0.0, -1e30)
        .astype(np.float32), "maskT_const")
    ident4_dram = nc.inline_tensor(np.eye(4, dtype=np.float32),
                                   "ident4_const")
    ident32_dram = nc.inline_tensor(np.eye(P, dtype=np.float32),
                                    "ident32_const")

    inv_sqrt_d = 1.0 / float(np.sqrt(D))

    with tile.TileContext(nc) as tc:
        with (
            tc.tile_pool(name="const", bufs=1) as constp,
            tc.tile_pool(name="warm", bufs=1) as warmp,
            tc.tile_pool(name="wpool", bufs=1) as wpool,
            tc.tile_pool(name="xtbp", bufs=1) as xtbp,
            tc.tile_pool(name="xt8p", bufs=1) as xt8p,
            tc.tile_pool(name="persist", bufs=1) as persist,
            tc.tile_pool(name="ptp", bufs=1) as ptpool,
            tc.tile_pool(name="outp", bufs=4) as outp,
            tc.tile_pool(name="small", bufs=8) as smallp,
        ):
            # ---- PE warmup: no external deps; covers NEFF/DMA startup
            # and un-throttles the HAM clock gate before real work ----
            wsrc = warmp.tile([P, 512], bf16)
            nc.vector.memset(wsrc[:], 0.0)
            with tc.tile_pool(name="wps", bufs=1, space="PSUM") as wpsp:
                wps = wpsp.tile([P, 512], f32)
                for _ in range(WARMUP_N):
                    nc.tensor.matmul(wps[:], wsrc[:, 0:P], wsrc[:],
                                     start=True, stop=True)

            maskT = constp.tile([P, P], f32)
            nc.scalar.dma_start(maskT[:], maskT_dram[:])
            ident4 = constp.tile([4, 4], f32)
            nc.scalar.dma_start(ident4[:], ident4_dram[:])
            ones8 = constp.tile([P, 1], f8)
            nc.vector.memset(ones8[:], 1.0)
            ones2 = constp.tile([P, 2, 16], f8)
            nc.vector.memset(ones2[:], 1.0)
            ident32 = constp.tile([P, P], f32)
            nc.scalar.dma_start(ident32[:], ident32_dram[:])
            biasm3 = constp.tile([P, 1], f32)
            nc.vector.memset(biasm3[:], -3.0)

            # ---- input DMA schedule (gpsimd SWDGE, casting) ----
            # X f32 -> bf16 SBUF, one DMA per 512-row chunk; weights
            # f32 -> fp8 SBUF, one DMA each via AP rearrange.  Chunk 0
            # first: the transpose->projection chain hangs off it.
            xbf = xtbp.tile([P, ST, D], f32, name="xbf")
            w_f8 = {}
            for name in ("q", "v", "k"):
                w_f8[name] = wpool.tile([P, DT, U], f8, name=f"w8_{name}")

            def load_w(name, ext):
                nc.gpsimd.dma_start(
                    w_f8[name][:],
                    ext[:].rearrange("(t p) u -> p t u", p=P))

            # X as plain f32 tiles on the two HWDGE queues (full
            # bandwidth, no cast bottleneck); weights stream through the
            # gpsimd SWDGE casting path in parallel
            load_w("q", wq_ext)
            load_w("v", wv_ext)
            load_w("k", wk_ext)
            for st in range(ST):
                qeng = nc.sync if st % 2 == 0 else nc.scalar
                qeng.dma_start(xbf[:, st, :],
                               x_ext[st * P:(st + 1) * P, :])

            xt8 = xt8p.tile([P, DT, S], f8, name="xt8")

            qT = persist.tile([P, UT, S], f8, name="qT")
            kT = persist.tile([P, UT, S], f8, name="kT")
            vt = persist.tile([P, ST, U], f8, name="vt")

            # ---- phase 1: x^T transposes + projections (fp8 DR) ----
            from contextlib import ExitStack
            spA_es = ExitStack()
            spsumA = spA_es.enter_context(
                tc.tile_pool(name="spsumA", bufs=1, space="PSUM"))
            p1_es = ExitStack()
            tp_es = ExitStack()
            ppsum = p1_es.enter_context(
                tc.tile_pool(name="ppsum", bufs=2, space="PSUM"))
            tpsum = tp_es.enter_context(
                tc.tile_pool(name="tpsum", bufs=2, space="PSUM"))
            if True:
                def emit_proj(c):
                    cs = slice(c * 512, (c + 1) * 512)
                    # bf16 PE transposes; 4 s-tiles share one PSUM bank
                    # per d, one DVE eviction each (casting to fp8)
                    for d in range(DT):
                        tb = tpsum.tile([P, 512], f32, tag="tpsum")
                        for k in range(4):
                            st = 4 * c + k
                            nc.tensor.transpose(
                                tb[:, k * P:(k + 1) * P],
                                xbf[:, st, d * P:(d + 1) * P],
                                ident32[:])
                        nc.vector.tensor_copy(out=xt8[:, d, cs], in_=tb[:])
                    # Q and K: [128,1024] two-bank PSUM, paired u-tiles
                    for dst, wkey in ((qT, "q"), (kT, "v")):
                        for up in range(UT // 2):
                            ps = ppsum.tile([P, 1024], f32, tag="ppsum")
                            for h in range(2):
                                u = up * 2 + h
                                for ki in range(0, DT, 2):
                                    nc.tensor.matmul(
                                        ps[:, h * 512:h * 512 + 512],
                                        w_f8[wkey][:, ki:ki + 2,
                                                   u * P:(u + 1) * P],
                                        xt8[:, ki:ki + 2, cs],
                                        start=(ki == 0),
                                        stop=(ki == DT - 2),
                                        perf_mode=DR)
                            nc.scalar.activation(
                                out=dst[:, up * 2:up * 2 + 2, cs],
                                in_=ps[:], func=AF.Sigmoid)
                    # V: paired s-tiles [128,1024]
                    for sp in range(2):
                        ps = ppsum.tile([P, 1024], f32, tag="ppsum")
                        for h in range(2):
                            st = c * 4 + sp * 2 + h
                            for ki in range(0, DT, 2):
                                nc.tensor.matmul(
                                    ps[:, h * 512:h * 512 + 512],
                                    xt8[:, ki:ki + 2,
                                        st * P:(st + 1) * P],
                                    w_f8["k"][:, ki:ki + 2, :],
                                    start=(ki == 0), stop=(ki == DT - 2),
                                    perf_mode=DR)
                        st0 = c * 4 + sp * 2
                        nc.scalar.activation(
                            out=vt[:, st0:st0 + 2, :], in_=ps[:],
                            func=AF.Sigmoid)

                for c in range(SC):
                    emit_proj(c)

            # free tpsum's two banks for the first score pool
            tp_es.close()

            # ---- phase 2: transposed-score attention ----
            # pT[:, j, q] holds P^T for key-tile j (valid q >= j*128)
            pT = ptpool.tile([P, ST, S], f8, name="pT")
            recipT = smallp.tile([P, 16], f32, name="recipT")
            ovals = [outp.tile([P, U], f32, tag="ov", name=f"ov{i}")
                     for i in range(ST)]

            sp_state = {"n": 0, "B": None}

            def spsum_tile():
                # alternate the two single-buffer score pools; pool B
                # only exists once ppsum's banks are released
                n = sp_state["n"]
                sp_state["n"] += 1
                pool = spsumA if n % 2 == 0 else sp_state["B"]
                return pool.tile([P, 1024], f32, tag="spsum",
                                 name=f"sps{n}")

            if True:
                recip_ps = None
                dps_cur = [None]

                def emit_scoresT(j, only_first_pair=False,
                                 skip_first_pair=False):
                    # queries [j*128, 2048), 512-aligned chunk ends
                    qlo = j * P
                    c0 = qlo // 512
                    # process chunks in [128,1024] two-bank psum pairs
                    for pidx, cp_lo in enumerate(range(c0, SC, 2)):
                        if only_first_pair and pidx > 0:
                            break
                        if skip_first_pair and pidx == 0:
                            continue
                        nch = min(2, SC - cp_lo)
                        ps = spsum_tile()
                        lo0 = max(qlo, cp_lo * 512)
                        for up in range(UT // 2):
                            for h in range(nch):
                                c = cp_lo + h
                                lo = max(qlo, c * 512)
                                w = 512 * (c + 1) - lo
                                po = h * 512 + (lo - c * 512)
                                nc.tensor.matmul(
                                    ps[:, po:po + w],
                                    kT[:, 2 * up:2 * up + 2,
                                       j * P:(j + 1) * P],
                                    qT[:, 2 * up:2 * up + 2, lo:lo + w],
                                    start=(up == 0), stop=(up == 1),
                                    perf_mode=DR)
                        # strict-causal mask on the diagonal block
                        if lo0 == qlo and cp_lo == c0:
                            po = lo0 - cp_lo * 512
                            nc.vector.tensor_add(
                                out=ps[:, po:po + P],
                                in0=ps[:, po:po + P], in1=maskT[:])
                        # single exp over the contiguous span of the pair
                        wtot = 512 * (cp_lo + nch) - lo0
                        po0 = lo0 - cp_lo * 512
                        nc.scalar.activation(
                            out=pT[:, j, lo0:lo0 + wtot],
                            in_=ps[:, po0:po0 + wtot], func=AF.Exp,
                            scale=inv_sqrt_d, bias=biasm3[:, 0:1])

                def emit_pv(i):
                    po = opsum.tile([P, U], f32, tag="opsum")
                    npair = (i + 1) // 2
                    odd = (i + 1) % 2
                    for jp in range(npair):
                        nc.tensor.matmul(
                            po[:], pT[:, 2 * jp:2 * jp + 2,
                                      i * P:(i + 1) * P],
                            vt[:, 2 * jp:2 * jp + 2, :],
                            start=(jp == 0), stop=(jp == npair - 1 and
                                                   not odd),
                            perf_mode=DR)
                    if odd:
                        nc.tensor.matmul(
                            po[:], pT[:, i, i * P:(i + 1) * P],
                            vt[:, i, :], start=(i == 0), stop=True)
                    nc.vector.tensor_copy(out=ovals[i][:], in_=po[:])

                def emit_denoms(c, jhi, fresh, close):
                    # denom[q] over queries [512c, 512c+512) from key
                    # tiles `fresh` (all contributing j <= jhi); ones
                    # stationary, DoubleRow sums tile pairs where the
                    # spans align.
                    if fresh and fresh[0] == 0:
                        dps_cur[0] = dpsum.tile([1, 512], f32, tag="dpsum",
                                                name=f"dps{c}_{jhi}")
                    dps = dps_cur[0]
                    first = bool(fresh) and fresh[0] == 0
                    k = 0
                    while k < len(fresh):
                        jj = fresh[k]
                        lo = max(jj * P, c * 512)
                        w = 512 * (c + 1) - lo
                        last = k + 1 >= len(fresh)
                        # pair jj, jj+1 when both have the full span
                        if (not last and fresh[k + 1] == jj + 1
                                and (jj + 1) * P <= c * 512):
                            nc.tensor.matmul(
                                dps[0:1, lo - c * 512:lo - c * 512 + w],
                                ones2[:, :, 0:1],
                                pT[:, jj:jj + 2, lo:lo + w],
                                start=(first and k == 0),
                                stop=(close and k + 2 >= len(fresh)),
                                perf_mode=DR)
                            k += 2
                        else:
                            nc.tensor.matmul(
                                dps[0:1, lo - c * 512:lo - c * 512 + w],
                                ones8[:, 0:1], pT[:, jj, lo:lo + w],
                                start=(first and k == 0),
                                stop=(close and last))
                            k += 1

                def emit_norm(c, rows):
                    # flip denoms to per-partition scalars, reciprocal,
                    # normalize + DMA the given rows of chunk c
                    dps = dps_cur[0]
                    dsb = smallp.tile([1, 512], f32, tag="dsb")
                    r0, r1 = rows[0] - 4 * c, rows[-1] - 4 * c + 1
                    nc.vector.tensor_copy(out=dsb[0:1, r0 * P:r1 * P],
                                          in_=dps[0:1, r0 * P:r1 * P])
                    for i in rows:
                        r = i - 4 * c
                        nc.tensor.transpose(
                            recip_ps[:, i:i + 1],
                            dsb[0:1, r * P:(r + 1) * P], ident4[0:1, 0:1])
                    rv = smallp.tile([P, len(rows)], f32, tag="rv")
                    # +1e-30 guard keeps fully-masked row 0 at output 0
                    nc.vector.tensor_scalar_add(
                        rv[:], recip_ps[:, rows[0]:rows[-1] + 1], 1e-30)
                    nc.vector.reciprocal(recipT[:, rows[0]:rows[-1] + 1],
                                         rv[:])
                    for i in rows:
                        ot = outp.tile([P, U], f32, tag="out")
                        nc.vector.tensor_scalar_mul(
                            ot[:], ovals[i][:], recipT[:, i:i + 1])
                        qeng = nc.sync if i % 2 == 0 else nc.scalar
                        qeng.dma_start(out_ext[i * P:(i + 1) * P, :],
                                       ot[:])

                # first chunk-pair of scoresT(0) runs off spsumA while
                # the phase-1 sigmoids drain; then release ppsum's banks
                # and open the remaining phase-2 pools
                emit_scoresT(0, only_first_pair=True)
                p1_es.close()
                p2_es = ExitStack()
                sp_state["B"] = p2_es.enter_context(
                    tc.tile_pool(name="spsumB", bufs=1, space="PSUM"))
                dpsum = p2_es.enter_context(
                    tc.tile_pool(name="dpsum", bufs=1, space="PSUM"))
                opsum = p2_es.enter_context(
                    tc.tile_pool(name="opsum", bufs=2, space="PSUM"))
                rpsum = p2_es.enter_context(
                    tc.tile_pool(name="rpsum", bufs=1, space="PSUM"))
                recip_ps = rpsum.tile([P, 16], f32)
                emit_scoresT(0, skip_first_pair=True)
                for j in range(ST):
                    if j + 1 < ST:
                        emit_scoresT(j + 1)
                    emit_pv(j)
                    c = j // 4
                    if j % 4 == 0:
                        emit_denoms(c, j, list(range(0, j + 1)), j == 15)
                    elif c < 3:
                        emit_denoms(c, j, [j], j % 4 == 3)
                    elif j == 14:
                        # rows 12-14 see no keys >= 1920: close early
                        emit_denoms(3, 14, [13, 14], False)
                        emit_norm(3, [12, 13, 14])
                    elif j == 15:
                        emit_denoms(3, 15, [15], True)
                        emit_norm(3, [15])
                    if j % 4 == 3 and c < 3:
                        emit_norm(c, [4 * c, 4 * c + 1, 4 * c + 2,
                                      4 * c + 3])
                p2_es.close()
                spA_es.close()

    nc.compile()
    return nc


def _get_nc():
    if "nc" not in _cache:
        _cache["nc"] = _build()
    return _cache["nc"]


def kernel(query, Wq, Wv, Wk):
    from concourse.bass_utils import run_bass_kernel_spmd

    nc = _get_nc()
    query = np.ascontiguousarray(query, dtype=np.float32)
    Wq = np.ascontiguousarray(Wq, dtype=np.float32)
    Wv = np.ascontiguousarray(Wv, dtype=np.float32)
    Wk = np.ascontiguousarray(Wk, dtype=np.float32)

    in_maps = [
        {"query": query[b], "Wq": Wq, "Wv": Wv, "Wk": Wk} for b in range(B)
    ]
    res = run_bass_kernel_spmd(nc, in_maps, core_ids=list(range(NCORES)))
    out = np.stack([np.asarray(res.results[b]["out"]) for b in range(B)])
    return out.astype(np.float32)
